# revision 9
# baseline (speedup 1.0000x reference)
"""Color-preserving non-local block (dense softmax attention, N=9216, I=32)
distributed over 8 TRN2 NeuronCores.

Sharding: data-parallel over batch B=2 (4 cores per batch) x sequence-parallel
over the N=9216 query rows (2304 rows per core).  Each core receives the full
[C, N] image of its batch (rolled so its query slice starts at column 0 --
softmax over keys is permutation-invariant, so rolling the key axis is free),
computes the projections redundantly, and produces its [C, 2304] output slice.
No collectives are needed.

Per-core pipeline (all on one NeuronCore):
  prologue: DMA x + weights, project theta (q rows only), phi, g^T (per
            128-wide kv tile, with a ones-column appended for the softmax
            denominator), and the channel gate.
  main loop over q chunks (512) x kv tiles (128):
      QK:  St[kv,q] = phi_tile^T @ theta          (TensorE, PSUM)
      exp: E = exp(St / T)                        (ScalarE, one instr / 4 tiles)
      PV:  Y[0:32,q] += g_tile^T_aug^T @ E        (TensorE, PSUM accumulate;
           row 32 of Y accumulates the softmax denominator via the ones col)
      epilogue: divide by denominator, W-projection, gate+residual, DMA out.
"""

import sys

for _p in ("/opt/trn_rl_repo",):
    if _p not in sys.path:
        sys.path.insert(0, _p)

import numpy as np

import concourse.bass as bass
import concourse.tile as tile
from concourse import bacc, mybir
from concourse.bass import ts, ds
from concourse.bass_utils import run_bass_kernel_spmd

F32 = mybir.dt.float32
F32R = mybir.dt.float32r

B, C, H, W = 2, 64, 96, 96
N = H * W                    # 9216
I = 32                       # inter dim
NB = 16                      # bottleneck dim of the gate
NCORES = 8
CPB = NCORES // B            # cores per batch = 4
QPC = N // CPB               # 2304 query rows per core
KT = 128                     # kv tile (partition dim of PV contraction)
NKV = N // KT                # 72
GK = 4                       # kv tiles per St/exp group
NG = NKV // GK               # 18 groups
QCH = 512                    # q chunk (PSUM free dim)
TEMP = 1.5
PR = 0.8
USE_F32R = True


# Matmul-fed tensors are declared natively float32r (streams at 1 cyc/row for
# free dim >= 256; walrus requires producers to write f32r, not bitcasts).
MMDT = F32R if USE_F32R else F32


def _r(ap):
    return ap


def _chunks():
    out = []
    q = 0
    while q < QPC:
        out.append((q, min(QCH, QPC - q)))
        q += QCH
    return out


def _emit(tc, nc, dr, out_d):
    mm = nc.tensor.matmul
    with (
        tc.tile_pool(name="consts", bufs=1) as consts,
        tc.tile_pool(name="work", bufs=2) as work,
        tc.tile_pool(name="epool", bufs=3) as epool,
    ):
        # ---- persistent SBUF tensors -------------------------------------
        xb_sb = consts.tile([C, N], MMDT)
        thw_sb = consts.tile([C, I], MMDT)
        phw_sb = consts.tile([C, I], MMDT)
        gw_sb = consts.tile([C, I], MMDT)
        ww_sb = consts.tile([I, C], MMDT)
        c1w_sb = consts.tile([C, NB], F32)
        c1b_sb = consts.tile([NB, 1], F32)
        c2w_sb = consts.tile([NB, C], F32)
        nc2b_sb = consts.tile([C, 1], F32)

        theta_sb = consts.tile([I, QPC], MMDT)
        phi_sb = consts.tile([I, N], MMDT)
        gt_sb = consts.tile([128, NKV, I + 1], MMDT)   # [kv-in-tile, tile, i|ones]
        gate_sb = consts.tile([C, 1], F32)            # 0.8 * sigmoid(...)
        pool_sb = consts.tile([C, 1], F32)            # sum_n x[c, n]
        h_sb = consts.tile([NB, 1], F32)
        eg_sb = consts.tile([C, 1], F32)

        nc.sync.dma_start(out=xb_sb, in_=dr["xb"])
        nc.sync.dma_start(out=thw_sb, in_=dr["thw"])
        nc.sync.dma_start(out=phw_sb, in_=dr["phw"])
        nc.sync.dma_start(out=gw_sb, in_=dr["gw"])
        nc.sync.dma_start(out=ww_sb, in_=dr["ww"])
        nc.sync.dma_start(out=c1w_sb, in_=dr["c1w"])
        nc.sync.dma_start(out=c1b_sb, in_=dr["c1b"])
        nc.sync.dma_start(out=c2w_sb, in_=dr["c2w"])
        nc.sync.dma_start(out=nc2b_sb, in_=dr["nc2b"])
        # memset can't encode f32r; stage ones in f32 and copy-round into the
        # denominator column of each g^T tile
        ones72 = consts.tile([128, NKV], F32)
        nc.vector.memset(ones72, 1.0)
        nc.vector.tensor_copy(out=gt_sb[:, :, I], in_=ones72)

        # ---- prologue: projections + gate --------------------------------
        with tc.tile_pool(name="ppsum", bufs=2, space="PSUM") as pp:
            # theta for this core's q rows: [I, QPC]
            for qs, qn in _chunks():
                pt = pp.tile([128, QCH], F32, tag="pp")
                mm(out=pt[:I, :qn], lhsT=_r(thw_sb), rhs=_r(xb_sb[:, ds(qs, qn)]),
                   start=True, stop=True)
                nc.vector.tensor_copy(out=theta_sb[:, ds(qs, qn)], in_=pt[:I, :qn])
            # phi over all kv: [I, N]
            for c in range(N // QCH):
                pt = pp.tile([128, QCH], F32, tag="pp")
                mm(out=pt[:I, :], lhsT=_r(phw_sb), rhs=_r(xb_sb[:, ts(c, QCH)]),
                   start=True, stop=True)
                nc.vector.tensor_copy(out=phi_sb[:, ts(c, QCH)], in_=pt[:I, :])
            # g^T per kv tile: gt[kv, t, i] = sum_c x[c, t*128+kv] g_w[i, c]
            done = 0
            while done < NKV:
                nt = min(16, NKV - done)
                pt = pp.tile([128, QCH], F32, tag="pp")
                for k in range(nt):
                    t = done + k
                    mm(out=pt[:, ts(k, I)], lhsT=_r(xb_sb[:, ts(t, KT)]),
                       rhs=_r(gw_sb), start=True, stop=True)
                nc.vector.tensor_copy(
                    out=gt_sb[:, done : done + nt, :I],
                    in_=pt[:, : nt * I].rearrange("p (k i) -> p k i", i=I),
                )
                done += nt

            # channel gate: 0.8 * sigmoid(cg2 @ relu(cg1 @ mean + b1) + b2)
            nc.vector.reduce_sum(out=pool_sb, in_=xb_sb, axis=mybir.AxisListType.X)
            pt = pp.tile([128, QCH], F32, tag="pp")
            mm(out=pt[:NB, 0:1], lhsT=_r(c1w_sb), rhs=_r(pool_sb), start=True, stop=True)
            nc.scalar.activation(out=h_sb, in_=pt[:NB, 0:1],
                                 func=mybir.ActivationFunctionType.Relu,
                                 bias=c1b_sb, scale=1.0 / float(N))
            pt2 = pp.tile([128, QCH], F32, tag="pp")
            mm(out=pt2[:C, 0:1], lhsT=_r(c2w_sb), rhs=_r(h_sb), start=True, stop=True)
            # sigmoid(z) = 1 / (1 + exp(-z));  exp is the table set the main
            # loop needs anyway, so avoid loading the sigmoid table.
            nc.scalar.activation(out=eg_sb, in_=pt2[:C, 0:1],
                                 func=mybir.ActivationFunctionType.Exp,
                                 bias=nc2b_sb, scale=-1.0)
            nc.vector.tensor_scalar_add(gate_sb, eg_sb, 1.0)
            nc.vector.reciprocal(out=gate_sb, in_=gate_sb)
            nc.vector.tensor_scalar_mul(gate_sb, gate_sb, PR)

        # ---- main loop ----------------------------------------------------
        with (
            tc.tile_pool(name="pst", bufs=1, space="PSUM") as pst,
            tc.tile_pool(name="py", bufs=2, space="PSUM") as py,
            tc.tile_pool(name="po", bufs=1, space="PSUM") as po,
        ):
            for qs, qn in _chunks():
                y_ps = py.tile([128, QCH], F32, tag="y")
                for g in range(NG):
                    st = pst.tile([128, GK, QCH], F32, tag="st")
                    for j in range(GK):
                        t = g * GK + j
                        mm(out=st[:, j, :qn],
                           lhsT=_r(phi_sb[:, ts(t, KT)]),
                           rhs=_r(theta_sb[:, ds(qs, qn)]),
                           start=True, stop=True)
                    e_t = epool.tile([128, GK, QCH], MMDT, tag="e")
                    nc.scalar.activation(out=e_t[:, :, :qn], in_=st[:, :, :qn],
                                         func=mybir.ActivationFunctionType.Exp,
                                         scale=1.0 / TEMP)
                    for j in range(GK):
                        t = g * GK + j
                        mm(out=y_ps[: I + 1, :qn],
                           lhsT=_r(gt_sb[:, t, :]),
                           rhs=_r(e_t[:, j, :qn]),
                           start=(t == 0), stop=(t == NKV - 1))
                # epilogue for this q chunk
                recip = work.tile([1, QCH], F32, tag="recip")
                nc.vector.reciprocal(out=recip[:, :qn], in_=y_ps[I : I + 1, :qn])
                bc = work.tile([I, QCH], F32, tag="bc")
                nc.gpsimd.partition_broadcast(bc[:, :qn], recip[:, :qn])
                yn = work.tile([I, QCH], MMDT, tag="yn")
                nc.vector.tensor_mul(yn[:, :qn], y_ps[:I, :qn], bc[:, :qn])
                o_ps = po.tile([C, QCH], F32, tag="o")
                mm(out=o_ps[:, :qn], lhsT=_r(ww_sb), rhs=_r(yn[:, :qn]),
                   start=True, stop=True)
                out_sb = work.tile([C, QCH], F32, tag="out")
                nc.vector.scalar_tensor_tensor(
                    out=out_sb[:, :qn], in0=o_ps[:, :qn], scalar=gate_sb,
                    in1=xb_sb[:, ds(qs, qn)].bitcast(F32),
                    op0=mybir.AluOpType.mult, op1=mybir.AluOpType.add)
                nc.sync.dma_start(out=out_d[:, ds(qs, qn)], in_=out_sb[:, :qn])


def build():
    nc = bacc.Bacc("TRN2", target_bir_lowering=False, debug=False)
    names = {
        "xb": [C, N], "thw": [C, I], "phw": [C, I], "gw": [C, I], "ww": [I, C],
        "c1w": [C, NB], "c1b": [NB, 1], "c2w": [NB, C], "nc2b": [C, 1],
    }
    dr = {k: nc.dram_tensor(k, shp, F32 if k in ("c1b", "nc2b", "c1w", "c2w") else MMDT,
                            kind="ExternalInput").ap()
          for k, shp in names.items()}
    out_d = nc.dram_tensor("out", [C, QPC], F32, kind="ExternalOutput").ap()
    with tile.TileContext(nc) as tc:
        _emit(tc, nc, dr, out_d)
    nc.compile()
    return nc


_NC = None


def _get_nc():
    global _NC
    if _NC is None:
        _NC = build()
    return _NC


def make_in_maps(inputs):
    xf = np.ascontiguousarray(np.asarray(inputs["x"], np.float32).reshape(B, C, N))
    shared = {
        "thw": np.ascontiguousarray(np.asarray(inputs["theta_w"], np.float32).T),
        "phw": np.ascontiguousarray(np.asarray(inputs["phi_w"], np.float32).T),
        "gw": np.ascontiguousarray(np.asarray(inputs["g_w"], np.float32).T),
        "ww": np.ascontiguousarray(np.asarray(inputs["W_w"], np.float32).T),
        "c1w": np.ascontiguousarray(np.asarray(inputs["cg1_w"], np.float32).T),
        "c1b": np.asarray(inputs["cg1_b"], np.float32).reshape(NB, 1).copy(),
        "c2w": np.ascontiguousarray(np.asarray(inputs["cg2_w"], np.float32).T),
        "nc2b": (-np.asarray(inputs["cg2_b"], np.float32)).reshape(C, 1).copy(),
    }
    in_maps = []
    for core in range(NCORES):
        b, q0 = core // CPB, (core % CPB) * QPC
        m = dict(shared)
        m["xb"] = np.ascontiguousarray(np.roll(xf[b], -q0, axis=1))
        in_maps.append(m)
    return in_maps


def gather(results):
    y = np.empty((B, C, N), np.float32)
    for core in range(NCORES):
        b, q0 = core // CPB, (core % CPB) * QPC
        y[b][:, q0 : q0 + QPC] = results[core]["out"]
    return y.reshape(B, C, H, W)


def run(inputs, trace=False, **kw):
    res = run_bass_kernel_spmd(_get_nc(), make_in_maps(inputs),
                               core_ids=list(range(NCORES)), trace=trace, **kw)
    return gather(res.results), res


def kernel(**inputs):
    out, _ = run(inputs)
    return out


# revision 12
# speedup vs baseline: 2.0577x; 2.0577x over previous
"""Color-preserving non-local block (dense softmax attention, N=9216, I=32)
distributed over 8 TRN2 NeuronCores.

Sharding: data-parallel over batch B=2 (4 cores per batch) x sequence-parallel
over the N=9216 query rows (2304 rows per core).  Each core receives the full
[C, N] image of its batch (rolled so its query slice starts at column 0 --
softmax over keys is permutation-invariant, so rolling the key axis is free),
computes the projections redundantly, and produces its [C, 2304] output slice.
No collectives are needed.

Per-core pipeline (v2 -- bf16 matmul operands, pair-packed tensor tiles):
  prologue: DMA x (f32 for the residual, bf16 for matmuls) + weights; project
            theta (query rows, replicated to partition rows 0-31 and 32-63),
            phi (pair-packed layout), g^T per 128-wide kv tile (with a ones
            column appended for the softmax denominator), and the channel gate.
  main loop over q chunks (512) x kv tile pairs:
      QK:  two row-tiled (K=32) matmuls -> St[kv, q] pair in PSUM
      exp: one ScalarE instr per pair: E = exp(St / T) -> bf16 SBUF
      PV:  two col-tiled (M=33) matmuls accumulate Y + denominator into PSUM
  epilogue per chunk: fold the two column groups, divide by the denominator,
  W-projection, channel-gate + residual, DMA out.
"""

import sys

for _p in ("/opt/trn_rl_repo",):
    if _p not in sys.path:
        sys.path.insert(0, _p)

import numpy as np
import ml_dtypes

import concourse.bass as bass
import concourse.tile as tile
from concourse import bacc, mybir
from concourse.bass import ts, ds
from concourse.bass_utils import run_bass_kernel_spmd

F32 = mybir.dt.float32
BF16 = mybir.dt.bfloat16

B, C, H, W = 2, 64, 96, 96
N = H * W                    # 9216
I = 32                       # inter dim
NB = 16                      # gate bottleneck dim
NCORES = 8
CPB = NCORES // B            # cores per batch = 4
QPC = N // CPB               # 2304 query rows per core
KT = 128                     # kv tile
NKV = N // KT                # 72
NP = NKV // 2                # 36 kv tile pairs
QCH = 512                    # q chunk (PSUM free dim)
GTS = 34                     # gt free stride (33 used, padded to keep 4B align)
TEMP = 1.5
PR = 0.8


def _chunks():
    out = []
    q = 0
    while q < QPC:
        out.append((q, min(QCH, QPC - q)))
        q += QCH
    return out


def _emit(tc, nc, dr, out_d):
    mm = nc.tensor.matmul
    with (
        tc.tile_pool(name="consts", bufs=1) as consts,
        tc.tile_pool(name="work", bufs=2) as work,
        tc.tile_pool(name="epool", bufs=3) as epool,
    ):
        # ---- persistent SBUF tensors -------------------------------------
        xb_sb = consts.tile([C, N], F32)       # residual + gate path
        xbh_sb = consts.tile([C, N], BF16)     # matmul path
        thw2_sb = consts.tile([C, 2 * I], BF16)
        phw_sb = consts.tile([C, I], BF16)
        gw_sb = consts.tile([C, I], BF16)
        ww_sb = consts.tile([I, C], BF16)
        c1w_sb = consts.tile([C, NB], F32)
        c1b_sb = consts.tile([NB, 1], F32)
        c2w_sb = consts.tile([NB, C], F32)
        nc2b_sb = consts.tile([C, 1], F32)

        # theta replicated on partitions 0-31 / 32-63 (row-tile operands)
        theta2_sb = consts.tile([2 * I, QPC], BF16)
        # phi pair-packed: [32j+i, p*128+m] = phi[i, (2p+j)*128+m]
        phip_sb = consts.tile([2 * I, N], BF16)
        gt_sb = consts.tile([128, NKV, GTS], BF16)  # [kv, tile, i | ones | pad]
        gate_sb = consts.tile([C, 1], F32)
        pool_sb = consts.tile([C, 1], F32)
        h_sb = consts.tile([NB, 1], F32)
        eg_sb = consts.tile([C, 1], F32)

        nc.sync.dma_start(out=xb_sb, in_=dr["xb"])
        nc.sync.dma_start(out=xbh_sb, in_=dr["xbh"])
        nc.sync.dma_start(out=thw2_sb, in_=dr["thw2"])
        nc.sync.dma_start(out=phw_sb, in_=dr["phw"])
        nc.sync.dma_start(out=gw_sb, in_=dr["gw"])
        nc.sync.dma_start(out=ww_sb, in_=dr["ww"])
        nc.sync.dma_start(out=c1w_sb, in_=dr["c1w"])
        nc.sync.dma_start(out=c1b_sb, in_=dr["c1b"])
        nc.sync.dma_start(out=c2w_sb, in_=dr["c2w"])
        nc.sync.dma_start(out=nc2b_sb, in_=dr["nc2b"])

        ones72 = consts.tile([128, NKV], F32)
        nc.vector.memset(ones72, 1.0)
        nc.vector.tensor_copy(out=gt_sb[:, :, I], in_=ones72)

        # ---- prologue projections ---------------------------------------
        with tc.tile_pool(name="ppsum", bufs=2, space="PSUM") as pp:
            # theta2: [64, QPC]; weight thw2 packs two copies of theta_w^T
            for qs, qn in _chunks():
                pt = pp.tile([128, QCH], F32, tag="pp")
                mm(out=pt[: 2 * I, :qn], lhsT=thw2_sb, rhs=xbh_sb[:, ds(qs, qn)],
                   start=True, stop=True)
                nc.vector.tensor_copy(out=theta2_sb[:, ds(qs, qn)],
                                      in_=pt[: 2 * I, :qn])
            # phip: per 512 free = 4 pair blocks; col tiles j=0,1
            for c in range(N // QCH):
                pt = pp.tile([128, QCH], F32, tag="pp")
                for j in range(2):
                    rhs = bass.AP(
                        tensor=xbh_sb.tensor, offset=xbh_sb.offset + j * KT,
                        ap=[xbh_sb.ap[0], [2 * KT, 4], [1, KT]],
                    )
                    mm(out=pt[32 * j : 32 * j + I, :], lhsT=phw_sb,
                       rhs=rhs, tile_position=(0, 32 * j), start=True, stop=True)
                nc.vector.tensor_copy(out=phip_sb[:, ts(c, QCH)],
                                      in_=pt[: 2 * I, :])
            # gt: per kv tile, x-tile-stationary projection of g
            done = 0
            while done < NKV:
                nt = min(16, NKV - done)
                pt = pp.tile([128, QCH], F32, tag="pp")
                for k in range(nt):
                    t = done + k
                    mm(out=pt[:, ts(k, I)], lhsT=xbh_sb[:, ts(t, KT)],
                       rhs=gw_sb, start=True, stop=True)
                nc.vector.tensor_copy(
                    out=gt_sb[:, done : done + nt, :I],
                    in_=pt[:, : nt * I].rearrange("p (k i) -> p k i", i=I),
                )
                done += nt

        # ---- main loop ---------------------------------------------------
        with (
            tc.tile_pool(name="pst", bufs=2, space="PSUM") as pst,
            tc.tile_pool(name="py", bufs=1, space="PSUM") as py,
            tc.tile_pool(name="po", bufs=1, space="PSUM") as po,
            tc.tile_pool(name="pg", bufs=1, space="PSUM") as pg,
        ):

            # ---- channel gate (off the critical path; consumed by the
            # chunk epilogues above -- Tile resolves the dependencies) ----
            nc.vector.reduce_sum(out=pool_sb, in_=xb_sb, axis=mybir.AxisListType.X)
            h_ps = pg.tile([128, 1], F32, tag="pg")
            mm(out=h_ps[:NB, :], lhsT=c1w_sb, rhs=pool_sb, start=True, stop=True)
            nc.scalar.activation(out=h_sb, in_=h_ps[:NB, :],
                                 func=mybir.ActivationFunctionType.Relu,
                                 bias=c1b_sb, scale=1.0 / float(N))
            z_ps = pg.tile([128, 1], F32, tag="pg")
            mm(out=z_ps[:C, :], lhsT=c2w_sb, rhs=h_sb, start=True, stop=True)
            # sigmoid(z) = 1/(1+exp(-z)); reuse the exp table set
            nc.scalar.activation(out=eg_sb, in_=z_ps[:C, :],
                                 func=mybir.ActivationFunctionType.Exp,
                                 bias=nc2b_sb, scale=-1.0)
            nc.vector.tensor_scalar_add(gate_sb, eg_sb, 1.0)
            nc.vector.reciprocal(out=gate_sb, in_=gate_sb)
            nc.vector.tensor_scalar_mul(gate_sb, gate_sb, PR)

            for qs, qn in _chunks():
                y_ps = py.tile([128, 2, QCH], F32, tag="y")
                for p in range(NP):
                    st = pst.tile([128, 2, QCH], F32, tag="st")
                    for j in range(2):
                        mm(out=st[:, j, :qn],
                           lhsT=phip_sb[32 * j : 32 * j + I, ts(p, KT)],
                           rhs=theta2_sb[32 * j : 32 * j + I, ds(qs, qn)],
                           tile_position=(32 * j, 0), start=True, stop=True)
                    e_t = epool.tile([128, 2, QCH], BF16, tag="e")
                    nc.scalar.activation(out=e_t[:, :, :qn], in_=st[:, :, :qn],
                                         func=mybir.ActivationFunctionType.Exp,
                                         scale=1.0 / TEMP)
                    for j in range(2):
                        t = 2 * p + j
                        mm(out=y_ps[64 * j : 64 * j + I + 1, j, :qn],
                           lhsT=gt_sb[:, t, : I + 1],
                           rhs=e_t[:, j, :qn],
                           tile_position=(0, 64 * j),
                           start=(t < 2), stop=(t >= NKV - 2))
                # epilogue for this q chunk
                ysb = work.tile([I + 1, QCH], F32, tag="ysb")
                nc.vector.tensor_copy(out=ysb[:, :qn],
                                      in_=y_ps[64 : 64 + I + 1, 1, :qn])
                ysum = work.tile([I + 1, QCH], F32, tag="ysum")
                nc.vector.tensor_add(ysum[:, :qn], y_ps[: I + 1, 0, :qn], ysb[:, :qn])
                recip = work.tile([1, QCH], F32, tag="recip")
                nc.vector.reciprocal(out=recip[:, :qn], in_=ysum[I : I + 1, :qn])
                bc = work.tile([I, QCH], F32, tag="bc")
                nc.gpsimd.partition_broadcast(bc[:, :qn], recip[:, :qn])
                yn = work.tile([I, QCH], BF16, tag="yn")
                nc.vector.tensor_mul(yn[:, :qn], ysum[:I, :qn], bc[:, :qn])
                o_ps = po.tile([C, QCH], F32, tag="o")
                mm(out=o_ps[:, :qn], lhsT=ww_sb, rhs=yn[:, :qn],
                   start=True, stop=True)
                out_sb = work.tile([C, QCH], F32, tag="out")
                nc.vector.scalar_tensor_tensor(
                    out=out_sb[:, :qn], in0=o_ps[:, :qn], scalar=gate_sb,
                    in1=xb_sb[:, ds(qs, qn)],
                    op0=mybir.AluOpType.mult, op1=mybir.AluOpType.add)
                nc.sync.dma_start(out=out_d[:, ds(qs, qn)], in_=out_sb[:, :qn])


def build():
    nc = bacc.Bacc("TRN2", target_bir_lowering=False, debug=False)
    names = {
        "xb": ([C, N], F32), "xbh": ([C, N], BF16),
        "thw2": ([C, 2 * I], BF16), "phw": ([C, I], BF16),
        "gw": ([C, I], BF16), "ww": ([I, C], BF16),
        "c1w": ([C, NB], F32), "c1b": ([NB, 1], F32),
        "c2w": ([NB, C], F32), "nc2b": ([C, 1], F32),
    }
    dr = {k: nc.dram_tensor(k, shp, dt, kind="ExternalInput").ap()
          for k, (shp, dt) in names.items()}
    out_d = nc.dram_tensor("out", [C, QPC], F32, kind="ExternalOutput").ap()
    with tile.TileContext(nc) as tc:
        _emit(tc, nc, dr, out_d)
    nc.compile()
    return nc


_NC = None


def _get_nc():
    global _NC
    if _NC is None:
        _NC = build()
    return _NC


def make_in_maps(inputs):
    bf = ml_dtypes.bfloat16
    xf = np.ascontiguousarray(np.asarray(inputs["x"], np.float32).reshape(B, C, N))
    thwT = np.asarray(inputs["theta_w"], np.float32).T        # [C, I]
    shared = {
        "thw2": np.ascontiguousarray(np.concatenate([thwT, thwT], 1)).astype(bf),
        "phw": np.ascontiguousarray(np.asarray(inputs["phi_w"], np.float32).T).astype(bf),
        "gw": np.ascontiguousarray(np.asarray(inputs["g_w"], np.float32).T).astype(bf),
        "ww": np.ascontiguousarray(np.asarray(inputs["W_w"], np.float32).T).astype(bf),
        "c1w": np.ascontiguousarray(np.asarray(inputs["cg1_w"], np.float32).T),
        "c1b": np.asarray(inputs["cg1_b"], np.float32).reshape(NB, 1).copy(),
        "c2w": np.ascontiguousarray(np.asarray(inputs["cg2_w"], np.float32).T),
        "nc2b": (-np.asarray(inputs["cg2_b"], np.float32)).reshape(C, 1).copy(),
    }
    in_maps = []
    for core in range(NCORES):
        b, q0 = core // CPB, (core % CPB) * QPC
        m = dict(shared)
        xr = np.ascontiguousarray(np.roll(xf[b], -q0, axis=1))
        m["xb"] = xr
        m["xbh"] = xr.astype(bf)
        in_maps.append(m)
    return in_maps


def gather(results):
    y = np.empty((B, C, N), np.float32)
    for core in range(NCORES):
        b, q0 = core // CPB, (core % CPB) * QPC
        y[b][:, q0 : q0 + QPC] = results[core]["out"]
    return y.reshape(B, C, H, W)


def run(inputs, trace=False, **kw):
    res = run_bass_kernel_spmd(_get_nc(), make_in_maps(inputs),
                               core_ids=list(range(NCORES)), trace=trace, **kw)
    return gather(res.results), res


def kernel(**inputs):
    out, _ = run(inputs)
    return out


# revision 13
# speedup vs baseline: 2.0748x; 1.0083x over previous
"""Color-preserving non-local block (dense softmax attention, N=9216, I=32)
distributed over 8 TRN2 NeuronCores.

Sharding: data-parallel over batch B=2 (4 cores per batch) x sequence-parallel
over the N=9216 query rows (2304 rows per core).  Each core receives the full
[C, N] image of its batch (rolled so its query slice starts at column 0 --
softmax over keys is permutation-invariant, so rolling the key axis is free),
computes the projections redundantly, and produces its [C, 2304] output slice.
No collectives are needed.

v3: every matmul uses a full K=128 contraction (K<128 streams at half clock on
this part).  theta/phi are projected with 4x-replicated weight matrices so the
QK matmul contracts over 4 redundant copies (St = 4x scores; the 1/4 folds
into the exp scale for free), x is sent twice-stacked on partitions for the
projections, and PV contracts over the 128-wide kv tile with a ones column
appended to g^T so the softmax denominator accumulates in PSUM row 32.
All matmuls are plain 128x128-mode (no tile_position -> no PE mode-switch
drains).  Per-chunk epilogues are deferred one chunk so the PE never waits on
the divide chain.

  main loop over q chunks (512) x kv tile pairs:
      QK:  2 plain matmuls  St[kv, q] = (phi4 tile)^T theta4     (233 ns each)
      exp: one ScalarE instr per pair: E = exp(St / (4 T)) -> bf16
      PV:  2 plain matmuls  Y[0:33, q] += gt_aug^T E   (PSUM accumulate)
"""

import sys

for _p in ("/opt/trn_rl_repo",):
    if _p not in sys.path:
        sys.path.insert(0, _p)

import numpy as np
import ml_dtypes

import concourse.bass as bass
import concourse.tile as tile
from concourse import bacc, mybir
from concourse.bass import ts, ds
from concourse.bass_utils import run_bass_kernel_spmd

F32 = mybir.dt.float32
BF16 = mybir.dt.bfloat16

B, C, H, W = 2, 64, 96, 96
N = H * W                    # 9216
I = 32                       # inter dim
NB = 16                      # gate bottleneck dim
NCORES = 8
CPB = NCORES // B            # cores per batch = 4
QPC = N // CPB               # 2304 query rows per core
KT = 128                     # kv tile
NKV = N // KT                # 72
NP = NKV // 2                # 36 kv tile pairs
QCH = 512                    # q chunk (PSUM free dim)
GTS = 34                     # gt free stride (33 used, kept 4B-aligned)
TEMP = 1.5
PR = 0.8


def _chunks():
    out = []
    q = 0
    while q < QPC:
        out.append((q, min(QCH, QPC - q)))
        q += QCH
    return out


def _emit(tc, nc, dr, out_d):
    mm = nc.tensor.matmul
    with (
        tc.tile_pool(name="consts", bufs=1) as consts,
        tc.tile_pool(name="work", bufs=2) as work,
        tc.tile_pool(name="epool", bufs=3) as epool,
    ):
        # ---- persistent SBUF tensors -------------------------------------
        xb_sb = consts.tile([C, N], F32)        # residual + gate path
        xbh2_sb = consts.tile([128, N], BF16)   # x stacked twice on partitions
        thw_sb = consts.tile([128, 128], BF16)  # 0.5 * theta_w^T tiled (2, 4)
        phw_sb = consts.tile([128, 128], BF16)  # 0.5 * phi_w^T tiled (2, 4)
        gw_sb = consts.tile([128, I], BF16)     # 0.5 * g_w^T tiled (2, 1)
        ww_sb = consts.tile([I, C], BF16)
        c1w_sb = consts.tile([C, NB], F32)
        c1b_sb = consts.tile([NB, 1], F32)
        c2w_sb = consts.tile([NB, C], F32)
        nc2b_sb = consts.tile([C, 1], F32)

        theta4_sb = consts.tile([128, QPC], BF16)   # theta replicated x4
        phi4_sb = consts.tile([128, N], BF16)       # phi replicated x4
        gt_sb = consts.tile([128, NKV, GTS], BF16)  # [kv, tile, i | ones | pad]
        gate_sb = consts.tile([C, 1], F32)
        pool_sb = consts.tile([C, 1], F32)
        h_sb = consts.tile([NB, 1], F32)
        eg_sb = consts.tile([C, 1], F32)

        nc.sync.dma_start(out=xb_sb, in_=dr["xb"])
        nc.sync.dma_start(out=xbh2_sb, in_=dr["xbh2"])
        nc.sync.dma_start(out=thw_sb, in_=dr["thw"])
        nc.sync.dma_start(out=phw_sb, in_=dr["phw"])
        nc.sync.dma_start(out=gw_sb, in_=dr["gw"])
        nc.sync.dma_start(out=ww_sb, in_=dr["ww"])
        nc.sync.dma_start(out=c1w_sb, in_=dr["c1w"])
        nc.sync.dma_start(out=c1b_sb, in_=dr["c1b"])
        nc.sync.dma_start(out=c2w_sb, in_=dr["c2w"])
        nc.sync.dma_start(out=nc2b_sb, in_=dr["nc2b"])

        ones72 = consts.tile([128, NKV], F32)
        nc.vector.memset(ones72, 1.0)
        nc.vector.tensor_copy(out=gt_sb[:, :, I], in_=ones72)

        # ---- prologue projections (all K=128) ----------------------------
        with tc.tile_pool(name="ppsum", bufs=2, space="PSUM") as pp:
            for qs, qn in _chunks():
                pt = pp.tile([128, QCH], F32, tag="pp")
                mm(out=pt[:, :qn], lhsT=thw_sb, rhs=xbh2_sb[:, ds(qs, qn)],
                   start=True, stop=True)
                nc.vector.tensor_copy(out=theta4_sb[:, ds(qs, qn)],
                                      in_=pt[:, :qn])
            for c in range(N // QCH):
                pt = pp.tile([128, QCH], F32, tag="pp")
                mm(out=pt, lhsT=phw_sb, rhs=xbh2_sb[:, ts(c, QCH)],
                   start=True, stop=True)
                nc.vector.tensor_copy(out=phi4_sb[:, ts(c, QCH)], in_=pt)
            done = 0
            while done < NKV:
                nt = min(16, NKV - done)
                pt = pp.tile([128, QCH], F32, tag="pp")
                for k in range(nt):
                    t = done + k
                    mm(out=pt[:, ts(k, I)], lhsT=xbh2_sb[:, ts(t, KT)],
                       rhs=gw_sb, start=True, stop=True)
                nc.vector.tensor_copy(
                    out=gt_sb[:, done : done + nt, :I],
                    in_=pt[:, : nt * I].rearrange("p (k i) -> p k i", i=I),
                )
                done += nt

        # ---- main loop ---------------------------------------------------
        with (
            tc.tile_pool(name="pst", bufs=2, space="PSUM") as pst,
            tc.tile_pool(name="py", bufs=2, space="PSUM") as py,
            tc.tile_pool(name="pmisc", bufs=1, space="PSUM") as pmisc,
        ):
            # channel gate (tiny; consumed by the deferred epilogues)
            nc.vector.reduce_sum(out=pool_sb, in_=xb_sb, axis=mybir.AxisListType.X)
            h_ps = pmisc.tile([128, QCH], F32, tag="m")
            mm(out=h_ps[:NB, 0:1], lhsT=c1w_sb, rhs=pool_sb, start=True, stop=True)
            nc.scalar.activation(out=h_sb, in_=h_ps[:NB, 0:1],
                                 func=mybir.ActivationFunctionType.Relu,
                                 bias=c1b_sb, scale=1.0 / float(N))
            z_ps = pmisc.tile([128, QCH], F32, tag="m")
            mm(out=z_ps[:C, 0:1], lhsT=c2w_sb, rhs=h_sb, start=True, stop=True)
            nc.scalar.activation(out=eg_sb, in_=z_ps[:C, 0:1],
                                 func=mybir.ActivationFunctionType.Exp,
                                 bias=nc2b_sb, scale=-1.0)
            nc.vector.tensor_scalar_add(gate_sb, eg_sb, 1.0)
            nc.vector.reciprocal(out=gate_sb, in_=gate_sb)
            nc.vector.tensor_scalar_mul(gate_sb, gate_sb, PR)

            pending = None
            for qs, qn in _chunks():
                if pending is not None:
                    pending()
                    pending = None
                y_ps = py.tile([128, QCH], F32, tag="y")
                for p in range(NP):
                    st = pst.tile([128, 2, QCH], F32, tag="st")
                    for j in range(2):
                        t = 2 * p + j
                        mm(out=st[:, j, :qn],
                           lhsT=phi4_sb[:, ts(t, KT)],
                           rhs=theta4_sb[:, ds(qs, qn)],
                           start=True, stop=True)
                    e_t = epool.tile([128, 2, QCH], BF16, tag="e")
                    nc.scalar.activation(out=e_t[:, :, :qn], in_=st[:, :, :qn],
                                         func=mybir.ActivationFunctionType.Exp,
                                         scale=1.0 / (4.0 * TEMP))
                    for j in range(2):
                        t = 2 * p + j
                        mm(out=y_ps[: I + 1, :qn],
                           lhsT=gt_sb[:, t, : I + 1],
                           rhs=e_t[:, j, :qn],
                           start=(t == 0), stop=(t == NKV - 1))
                # DVE part of the epilogue now; PE/DMA part deferred a chunk
                ysum = work.tile([I + 1, QCH], F32, tag="ysum")
                nc.vector.tensor_copy(out=ysum[:, :qn], in_=y_ps[: I + 1, :qn])
                recip = work.tile([1, QCH], F32, tag="recip")
                nc.vector.reciprocal(out=recip[:, :qn], in_=ysum[I : I + 1, :qn])
                bc = work.tile([I, QCH], F32, tag="bc")
                nc.gpsimd.partition_broadcast(bc[:, :qn], recip[:, :qn])
                yn = work.tile([I, QCH], BF16, tag="yn")
                nc.vector.tensor_mul(yn[:, :qn], ysum[:I, :qn], bc[:, :qn])

                def _tail(qs=qs, qn=qn, yn=yn):
                    o_ps = pmisc.tile([128, QCH], F32, tag="m")
                    mm(out=o_ps[:C, :qn], lhsT=ww_sb, rhs=yn[:, :qn],
                       start=True, stop=True)
                    out_sb = work.tile([C, QCH], F32, tag="out")
                    nc.vector.scalar_tensor_tensor(
                        out=out_sb[:, :qn], in0=o_ps[:C, :qn], scalar=gate_sb,
                        in1=xb_sb[:, ds(qs, qn)],
                        op0=mybir.AluOpType.mult, op1=mybir.AluOpType.add)
                    nc.sync.dma_start(out=out_d[:, ds(qs, qn)],
                                      in_=out_sb[:, :qn])

                pending = _tail
            pending()


def build():
    nc = bacc.Bacc("TRN2", target_bir_lowering=False, debug=False)
    names = {
        "xb": ([C, N], F32), "xbh2": ([128, N], BF16),
        "thw": ([128, 128], BF16), "phw": ([128, 128], BF16),
        "gw": ([128, I], BF16), "ww": ([I, C], BF16),
        "c1w": ([C, NB], F32), "c1b": ([NB, 1], F32),
        "c2w": ([NB, C], F32), "nc2b": ([C, 1], F32),
    }
    dr = {k: nc.dram_tensor(k, shp, dt, kind="ExternalInput").ap()
          for k, (shp, dt) in names.items()}
    out_d = nc.dram_tensor("out", [C, QPC], F32, kind="ExternalOutput").ap()
    with tile.TileContext(nc) as tc:
        _emit(tc, nc, dr, out_d)
    nc.compile()
    return nc


_NC = None


def _get_nc():
    global _NC
    if _NC is None:
        _NC = build()
    return _NC


def make_in_maps(inputs):
    bf = ml_dtypes.bfloat16
    xf = np.ascontiguousarray(np.asarray(inputs["x"], np.float32).reshape(B, C, N))
    thwT = np.asarray(inputs["theta_w"], np.float32).T        # [C, I]
    phwT = np.asarray(inputs["phi_w"], np.float32).T
    gwT = np.asarray(inputs["g_w"], np.float32).T
    shared = {
        "thw": np.ascontiguousarray(np.tile(thwT, (2, 4)) * 0.5).astype(bf),
        "phw": np.ascontiguousarray(np.tile(phwT, (2, 4)) * 0.5).astype(bf),
        "gw": np.ascontiguousarray(np.tile(gwT, (2, 1)) * 0.5).astype(bf),
        "ww": np.ascontiguousarray(np.asarray(inputs["W_w"], np.float32).T).astype(bf),
        "c1w": np.ascontiguousarray(np.asarray(inputs["cg1_w"], np.float32).T),
        "c1b": np.asarray(inputs["cg1_b"], np.float32).reshape(NB, 1).copy(),
        "c2w": np.ascontiguousarray(np.asarray(inputs["cg2_w"], np.float32).T),
        "nc2b": (-np.asarray(inputs["cg2_b"], np.float32)).reshape(C, 1).copy(),
    }
    in_maps = []
    for core in range(NCORES):
        b, q0 = core // CPB, (core % CPB) * QPC
        m = dict(shared)
        xr = np.ascontiguousarray(np.roll(xf[b], -q0, axis=1))
        m["xb"] = xr
        m["xbh2"] = np.ascontiguousarray(np.tile(xr, (2, 1))).astype(bf)
        in_maps.append(m)
    return in_maps


def gather(results):
    y = np.empty((B, C, N), np.float32)
    for core in range(NCORES):
        b, q0 = core // CPB, (core % CPB) * QPC
        y[b][:, q0 : q0 + QPC] = results[core]["out"]
    return y.reshape(B, C, H, W)


def run(inputs, trace=False, **kw):
    res = run_bass_kernel_spmd(_get_nc(), make_in_maps(inputs),
                               core_ids=list(range(NCORES)), trace=trace, **kw)
    return gather(res.results), res


def kernel(**inputs):
    out, _ = run(inputs)
    return out


# revision 15
# speedup vs baseline: 2.2568x; 1.0877x over previous
"""Color-preserving non-local block (dense softmax attention, N=9216, I=32)
distributed over 8 TRN2 NeuronCores.

Sharding: data-parallel over batch B=2 (4 cores per batch) x sequence-parallel
over the N=9216 query rows (2304 rows per core).  Each core receives the full
[C, N] image of its batch (rolled so its query slice starts at column 0 --
softmax over keys is permutation-invariant, so rolling the key axis is free),
computes the projections redundantly, and produces its [C, 2304] output slice.
No collectives are needed.

v3: every matmul uses a full K=128 contraction (K<128 streams at half clock on
this part).  theta/phi are projected with 4x-replicated weight matrices so the
QK matmul contracts over 4 redundant copies (St = 4x scores; the 1/4 folds
into the exp scale for free), x is sent twice-stacked on partitions for the
projections, and PV contracts over the 128-wide kv tile with a ones column
appended to g^T so the softmax denominator accumulates in PSUM row 32.
All matmuls are plain 128x128-mode (no tile_position -> no PE mode-switch
drains).  Per-chunk epilogues are deferred one chunk so the PE never waits on
the divide chain.

  main loop over q chunks (512) x kv tile pairs:
      QK:  2 plain matmuls  St[kv, q] = (phi4 tile)^T theta4     (233 ns each)
      exp: one ScalarE instr per pair: E = exp(St / (4 T)) -> bf16
      PV:  2 plain matmuls  Y[0:33, q] += gt_aug^T E   (PSUM accumulate)
"""

import sys

for _p in ("/opt/trn_rl_repo",):
    if _p not in sys.path:
        sys.path.insert(0, _p)

import numpy as np
import ml_dtypes

import concourse.bass as bass
import concourse.tile as tile
from concourse import bacc, mybir
from concourse.bass import ts, ds
from concourse.bass_utils import run_bass_kernel_spmd

F32 = mybir.dt.float32
BF16 = mybir.dt.bfloat16

B, C, H, W = 2, 64, 96, 96
N = H * W                    # 9216
I = 32                       # inter dim
NB = 16                      # gate bottleneck dim
NCORES = 8
CPB = NCORES // B            # cores per batch = 4
QPC = N // CPB               # 2304 query rows per core
KT = 128                     # kv tile
NKV = N // KT                # 72
GK = 3                       # kv tiles per St/exp group
NGR = NKV // GK              # 24 groups
QCH = 512                    # q chunk (PSUM free dim)
GTS = 34                     # gt free stride (33 used, kept 4B-aligned)
TEMP = 1.5
PR = 0.8


def _chunks():
    out = []
    q = 0
    while q < QPC:
        out.append((q, min(QCH, QPC - q)))
        q += QCH
    return out


def _emit(tc, nc, dr, out_d):
    mm = nc.tensor.matmul
    with (
        tc.tile_pool(name="consts", bufs=1) as consts,
        tc.tile_pool(name="work", bufs=2) as work,
        tc.tile_pool(name="epool", bufs=4) as epool,
    ):
        # ---- persistent SBUF tensors -------------------------------------
        xb_sb = consts.tile([C, N], F32)        # residual + gate path
        xbh2_sb = consts.tile([128, N], BF16)   # x stacked twice on partitions
        thw_sb = consts.tile([128, 128], BF16)  # 0.5 * theta_w^T tiled (2, 4)
        phw_sb = consts.tile([128, 128], BF16)  # 0.5 * phi_w^T tiled (2, 4)
        gw_sb = consts.tile([128, I], BF16)     # 0.5 * g_w^T tiled (2, 1)
        ww_sb = consts.tile([I, C], BF16)
        c1w_sb = consts.tile([C, NB], F32)
        c1b_sb = consts.tile([NB, 1], F32)
        c2w_sb = consts.tile([NB, C], F32)
        nc2b_sb = consts.tile([C, 1], F32)

        theta4_sb = consts.tile([128, QPC], BF16)   # theta replicated x4
        phi4_sb = consts.tile([128, N], BF16)       # phi replicated x4
        gt_sb = consts.tile([128, NKV, GTS], BF16)  # [kv, tile, i | ones | pad]
        gate_sb = consts.tile([C, 1], F32)
        pool_sb = consts.tile([C, 1], F32)
        h_sb = consts.tile([NB, 1], F32)
        eg_sb = consts.tile([C, 1], F32)

        nc.sync.dma_start(out=xb_sb, in_=dr["xb"])
        nc.sync.dma_start(out=xbh2_sb, in_=dr["xbh2"])
        nc.sync.dma_start(out=thw_sb, in_=dr["thw"])
        nc.sync.dma_start(out=phw_sb, in_=dr["phw"])
        nc.sync.dma_start(out=gw_sb, in_=dr["gw"])
        nc.sync.dma_start(out=ww_sb, in_=dr["ww"])
        nc.sync.dma_start(out=c1w_sb, in_=dr["c1w"])
        nc.sync.dma_start(out=c1b_sb, in_=dr["c1b"])
        nc.sync.dma_start(out=c2w_sb, in_=dr["c2w"])
        nc.sync.dma_start(out=nc2b_sb, in_=dr["nc2b"])

        ones72 = consts.tile([128, NKV], F32)
        nc.vector.memset(ones72, 1.0)
        nc.vector.tensor_copy(out=gt_sb[:, :, I], in_=ones72)

        # ---- prologue projections (all K=128) ----------------------------
        with tc.tile_pool(name="ppsum", bufs=2, space="PSUM") as pp:
            for qs, qn in _chunks():
                pt = pp.tile([128, QCH], F32, tag="pp")
                mm(out=pt[:, :qn], lhsT=thw_sb, rhs=xbh2_sb[:, ds(qs, qn)],
                   start=True, stop=True)
                nc.vector.tensor_copy(out=theta4_sb[:, ds(qs, qn)],
                                      in_=pt[:, :qn])
            for c in range(N // QCH):
                pt = pp.tile([128, QCH], F32, tag="pp")
                mm(out=pt, lhsT=phw_sb, rhs=xbh2_sb[:, ts(c, QCH)],
                   start=True, stop=True)
                nc.vector.tensor_copy(out=phi4_sb[:, ts(c, QCH)], in_=pt)
            done = 0
            while done < NKV:
                nt = min(16, NKV - done)
                pt = pp.tile([128, QCH], F32, tag="pp")
                for k in range(nt):
                    t = done + k
                    mm(out=pt[:, ts(k, I)], lhsT=xbh2_sb[:, ts(t, KT)],
                       rhs=gw_sb, start=True, stop=True)
                nc.vector.tensor_copy(
                    out=gt_sb[:, done : done + nt, :I],
                    in_=pt[:, : nt * I].rearrange("p (k i) -> p k i", i=I),
                )
                done += nt

        # ---- main loop ---------------------------------------------------
        with (
            tc.tile_pool(name="pst", bufs=2, space="PSUM") as pst,
            tc.tile_pool(name="py", bufs=1, space="PSUM") as py,
            tc.tile_pool(name="pmisc", bufs=1, space="PSUM") as pmisc,
        ):
            def emit_gate():
                # channel gate; emitted after chunk 0's pairs so its matmuls
                # (which wait on the DVE mean-reduce) never block the PE queue
                # ahead of the main stream
                nc.vector.reduce_sum(out=pool_sb, in_=xb_sb,
                                     axis=mybir.AxisListType.X)
                h_ps = pmisc.tile([128, QCH], F32, tag="m")
                mm(out=h_ps[:NB, 0:1], lhsT=c1w_sb, rhs=pool_sb,
                   start=True, stop=True)
                nc.scalar.activation(out=h_sb, in_=h_ps[:NB, 0:1],
                                     func=mybir.ActivationFunctionType.Relu,
                                     bias=c1b_sb, scale=1.0 / float(N))
                z_ps = pmisc.tile([128, QCH], F32, tag="m")
                mm(out=z_ps[:C, 0:1], lhsT=c2w_sb, rhs=h_sb,
                   start=True, stop=True)
                nc.scalar.activation(out=eg_sb, in_=z_ps[:C, 0:1],
                                     func=mybir.ActivationFunctionType.Exp,
                                     bias=nc2b_sb, scale=-1.0)
                nc.vector.tensor_scalar_add(gate_sb, eg_sb, 1.0)
                nc.vector.reciprocal(out=gate_sb, in_=gate_sb)
                nc.vector.tensor_scalar_mul(gate_sb, gate_sb, PR)

            pending = None
            for ci, (qs, qn) in enumerate(_chunks()):
                y_ps = py.tile([I + 1, QCH], F32, tag="y")
                for g in range(NGR):
                    # the previous chunk's PE tail goes here, a few groups in,
                    # so its divide chain has finished on DVE by now
                    if g == 4 and pending is not None:
                        pending()
                        pending = None
                    st = pst.tile([128, GK, QCH], F32, tag="st")
                    for j in range(GK):
                        t = GK * g + j
                        mm(out=st[:, j, :qn],
                           lhsT=phi4_sb[:, ts(t, KT)],
                           rhs=theta4_sb[:, ds(qs, qn)],
                           start=True, stop=True)
                    e_t = epool.tile([128, GK, QCH], BF16, tag="e")
                    nc.scalar.activation(out=e_t[:, :, :qn], in_=st[:, :, :qn],
                                         func=mybir.ActivationFunctionType.Exp,
                                         scale=1.0 / (4.0 * TEMP))
                    for j in range(GK):
                        t = GK * g + j
                        mm(out=y_ps[:, :qn],
                           lhsT=gt_sb[:, t, : I + 1],
                           rhs=e_t[:, j, :qn],
                           start=(t == 0), stop=(t == NKV - 1))
                if ci == 0:
                    emit_gate()
                # DVE part of the epilogue now (frees y); PE/DMA deferred
                ysum = work.tile([I + 1, QCH], F32, tag="ysum")
                nc.vector.tensor_copy(out=ysum[:, :qn], in_=y_ps[:, :qn])
                recip = work.tile([1, QCH], F32, tag="recip")
                nc.vector.reciprocal(out=recip[:, :qn],
                                     in_=ysum[I : I + 1, :qn])
                bc = work.tile([I, QCH], F32, tag="bc")
                nc.gpsimd.partition_broadcast(bc[:, :qn], recip[:, :qn])
                yn = work.tile([I, QCH], BF16, tag="yn")
                nc.vector.tensor_mul(yn[:, :qn], ysum[:I, :qn], bc[:, :qn])

                def _tail(qs=qs, qn=qn, yn=yn):
                    o_ps = pmisc.tile([128, QCH], F32, tag="m")
                    mm(out=o_ps[:C, :qn], lhsT=ww_sb, rhs=yn[:, :qn],
                       start=True, stop=True)
                    out_sb = work.tile([C, QCH], F32, tag="out")
                    nc.vector.scalar_tensor_tensor(
                        out=out_sb[:, :qn], in0=o_ps[:C, :qn], scalar=gate_sb,
                        in1=xb_sb[:, ds(qs, qn)],
                        op0=mybir.AluOpType.mult, op1=mybir.AluOpType.add)
                    nc.sync.dma_start(out=out_d[:, ds(qs, qn)],
                                      in_=out_sb[:, :qn])

                pending = _tail
            pending()


def build():
    nc = bacc.Bacc("TRN2", target_bir_lowering=False, debug=False)
    names = {
        "xb": ([C, N], F32), "xbh2": ([128, N], BF16),
        "thw": ([128, 128], BF16), "phw": ([128, 128], BF16),
        "gw": ([128, I], BF16), "ww": ([I, C], BF16),
        "c1w": ([C, NB], F32), "c1b": ([NB, 1], F32),
        "c2w": ([NB, C], F32), "nc2b": ([C, 1], F32),
    }
    dr = {k: nc.dram_tensor(k, shp, dt, kind="ExternalInput").ap()
          for k, (shp, dt) in names.items()}
    out_d = nc.dram_tensor("out", [C, QPC], F32, kind="ExternalOutput").ap()
    with tile.TileContext(nc) as tc:
        _emit(tc, nc, dr, out_d)
    nc.compile()
    return nc


_NC = None


def _get_nc():
    global _NC
    if _NC is None:
        _NC = build()
    return _NC


def make_in_maps(inputs):
    bf = ml_dtypes.bfloat16
    xf = np.ascontiguousarray(np.asarray(inputs["x"], np.float32).reshape(B, C, N))
    thwT = np.asarray(inputs["theta_w"], np.float32).T        # [C, I]
    phwT = np.asarray(inputs["phi_w"], np.float32).T
    gwT = np.asarray(inputs["g_w"], np.float32).T
    shared = {
        "thw": np.ascontiguousarray(np.tile(thwT, (2, 4)) * 0.5).astype(bf),
        "phw": np.ascontiguousarray(np.tile(phwT, (2, 4)) * 0.5).astype(bf),
        "gw": np.ascontiguousarray(np.tile(gwT, (2, 1)) * 0.5).astype(bf),
        "ww": np.ascontiguousarray(np.asarray(inputs["W_w"], np.float32).T).astype(bf),
        "c1w": np.ascontiguousarray(np.asarray(inputs["cg1_w"], np.float32).T),
        "c1b": np.asarray(inputs["cg1_b"], np.float32).reshape(NB, 1).copy(),
        "c2w": np.ascontiguousarray(np.asarray(inputs["cg2_w"], np.float32).T),
        "nc2b": (-np.asarray(inputs["cg2_b"], np.float32)).reshape(C, 1).copy(),
    }
    in_maps = []
    for core in range(NCORES):
        b, q0 = core // CPB, (core % CPB) * QPC
        m = dict(shared)
        xr = np.ascontiguousarray(np.roll(xf[b], -q0, axis=1))
        m["xb"] = xr
        m["xbh2"] = np.ascontiguousarray(np.tile(xr, (2, 1))).astype(bf)
        in_maps.append(m)
    return in_maps


def gather(results):
    y = np.empty((B, C, N), np.float32)
    for core in range(NCORES):
        b, q0 = core // CPB, (core % CPB) * QPC
        y[b][:, q0 : q0 + QPC] = results[core]["out"]
    return y.reshape(B, C, H, W)


def run(inputs, trace=False, **kw):
    res = run_bass_kernel_spmd(_get_nc(), make_in_maps(inputs),
                               core_ids=list(range(NCORES)), trace=trace, **kw)
    return gather(res.results), res


def kernel(**inputs):
    out, _ = run(inputs)
    return out


# revision 16
# speedup vs baseline: 2.3121x; 1.0245x over previous
"""Color-preserving non-local block (dense softmax attention, N=9216, I=32)
distributed over 8 TRN2 NeuronCores.

Sharding: data-parallel over batch B=2 (4 cores per batch) x sequence-parallel
over the N=9216 query rows (2304 rows per core).  Each core receives the full
[C, N] image of its batch (rolled so its query slice starts at column 0 --
softmax over keys is permutation-invariant, so rolling the key axis is free),
computes the projections redundantly, and produces its [C, 2304] output slice.
No collectives are needed.

v3: every matmul uses a full K=128 contraction (K<128 streams at half clock on
this part).  theta/phi are projected with 4x-replicated weight matrices so the
QK matmul contracts over 4 redundant copies (St = 4x scores; the 1/4 folds
into the exp scale for free), x is sent twice-stacked on partitions for the
projections, and PV contracts over the 128-wide kv tile with a ones column
appended to g^T so the softmax denominator accumulates in PSUM row 32.
All matmuls are plain 128x128-mode (no tile_position -> no PE mode-switch
drains).  Per-chunk epilogues are deferred one chunk so the PE never waits on
the divide chain.

  main loop over q chunks (512) x kv tile pairs:
      QK:  2 plain matmuls  St[kv, q] = (phi4 tile)^T theta4     (233 ns each)
      exp: one ScalarE instr per pair: E = exp(St / (4 T)) -> bf16
      PV:  2 plain matmuls  Y[0:33, q] += gt_aug^T E   (PSUM accumulate)
"""

import sys

for _p in ("/opt/trn_rl_repo",):
    if _p not in sys.path:
        sys.path.insert(0, _p)

import numpy as np
import ml_dtypes

import concourse.bass as bass
import concourse.tile as tile
from concourse import bacc, mybir
from concourse.bass import ts, ds
from concourse.bass_utils import run_bass_kernel_spmd

F32 = mybir.dt.float32
BF16 = mybir.dt.bfloat16

B, C, H, W = 2, 64, 96, 96
N = H * W                    # 9216
I = 32                       # inter dim
NB = 16                      # gate bottleneck dim
NCORES = 8
CPB = NCORES // B            # cores per batch = 4
QPC = N // CPB               # 2304 query rows per core
KT = 128                     # kv tile
NKV = N // KT                # 72
GK = 3                       # kv tiles per St/exp group
NGR = NKV // GK              # 24 groups
QCH = 512                    # q chunk (PSUM free dim)
GTS = 34                     # gt free stride (33 used, kept 4B-aligned)
TEMP = 1.5
PR = 0.8


def _chunks():
    out = []
    q = 0
    while q < QPC:
        out.append((q, min(QCH, QPC - q)))
        q += QCH
    return out


def _emit(tc, nc, dr, out_d):
    mm = nc.tensor.matmul
    with (
        tc.tile_pool(name="consts", bufs=1) as consts,
        tc.tile_pool(name="work", bufs=2) as work,
        tc.tile_pool(name="epool", bufs=4) as epool,
    ):
        # ---- persistent SBUF tensors -------------------------------------
        xb_sb = consts.tile([C, N], F32)        # residual + gate path
        xbh2_sb = consts.tile([128, N], BF16)   # x stacked twice on partitions
        thw_sb = consts.tile([128, 128], BF16)  # 0.5 * theta_w^T tiled (2, 4)
        phw_sb = consts.tile([128, 128], BF16)  # 0.5 * phi_w^T tiled (2, 4)
        gw_sb = consts.tile([128, I], BF16)     # 0.5 * g_w^T tiled (2, 1)
        ww_sb = consts.tile([I, C], BF16)
        c1w_sb = consts.tile([C, NB], F32)
        c1b_sb = consts.tile([NB, 1], F32)
        c2w_sb = consts.tile([NB, C], F32)
        nc2b_sb = consts.tile([C, 1], F32)

        theta4_sb = consts.tile([128, QPC], BF16)   # theta replicated x4
        phi4_sb = consts.tile([128, N], BF16)       # phi replicated x4
        gt_sb = consts.tile([128, NKV, GTS], BF16)  # [kv, tile, i | ones | pad]
        gate_sb = consts.tile([C, 1], F32)
        pool_sb = consts.tile([C, 1], F32)
        h_sb = consts.tile([NB, 1], F32)
        eg_sb = consts.tile([C, 1], F32)

        nc.sync.dma_start(out=xb_sb, in_=dr["xb"])
        nc.sync.dma_start(out=xbh2_sb, in_=dr["xbh2"])
        nc.sync.dma_start(out=thw_sb, in_=dr["thw"])
        nc.sync.dma_start(out=phw_sb, in_=dr["phw"])
        nc.sync.dma_start(out=gw_sb, in_=dr["gw"])
        nc.sync.dma_start(out=ww_sb, in_=dr["ww"])
        nc.sync.dma_start(out=c1w_sb, in_=dr["c1w"])
        nc.sync.dma_start(out=c1b_sb, in_=dr["c1b"])
        nc.sync.dma_start(out=c2w_sb, in_=dr["c2w"])
        nc.sync.dma_start(out=nc2b_sb, in_=dr["nc2b"])

        ones72 = consts.tile([128, NKV], F32)
        nc.vector.memset(ones72, 1.0)
        nc.vector.tensor_copy(out=gt_sb[:, :, I], in_=ones72)

        # ---- prologue projections (all K=128) ----------------------------
        with tc.tile_pool(name="ppsum", bufs=4, space="PSUM") as pp:
            for qs, qn in _chunks():
                pt = pp.tile([128, QCH], F32, tag="pp")
                mm(out=pt[:, :qn], lhsT=thw_sb, rhs=xbh2_sb[:, ds(qs, qn)],
                   start=True, stop=True)
                nc.scalar.copy(out=theta4_sb[:, ds(qs, qn)], in_=pt[:, :qn])
            for c in range(N // QCH):
                pt = pp.tile([128, QCH], F32, tag="pp")
                mm(out=pt, lhsT=phw_sb, rhs=xbh2_sb[:, ts(c, QCH)],
                   start=True, stop=True)
                nc.scalar.copy(out=phi4_sb[:, ts(c, QCH)], in_=pt)
            done = 0
            while done < NKV:
                nt = min(16, NKV - done)
                pt = pp.tile([128, QCH], F32, tag="pp")
                for k in range(nt):
                    t = done + k
                    mm(out=pt[:, ts(k, I)], lhsT=xbh2_sb[:, ts(t, KT)],
                       rhs=gw_sb, start=True, stop=True)
                nc.vector.tensor_copy(
                    out=gt_sb[:, done : done + nt, :I],
                    in_=pt[:, : nt * I].rearrange("p (k i) -> p k i", i=I),
                )
                done += nt

        # ---- main loop ---------------------------------------------------
        with (
            tc.tile_pool(name="pst", bufs=2, space="PSUM") as pst,
            tc.tile_pool(name="py", bufs=1, space="PSUM") as py,
            tc.tile_pool(name="pmisc", bufs=1, space="PSUM") as pmisc,
        ):
            def emit_gate():
                # channel gate; emitted after chunk 0's pairs so its matmuls
                # (which wait on the DVE mean-reduce) never block the PE queue
                # ahead of the main stream
                nc.vector.reduce_sum(out=pool_sb, in_=xb_sb,
                                     axis=mybir.AxisListType.X)
                h_ps = pmisc.tile([128, QCH], F32, tag="m")
                mm(out=h_ps[:NB, 0:1], lhsT=c1w_sb, rhs=pool_sb,
                   start=True, stop=True)
                nc.scalar.activation(out=h_sb, in_=h_ps[:NB, 0:1],
                                     func=mybir.ActivationFunctionType.Relu,
                                     bias=c1b_sb, scale=1.0 / float(N))
                z_ps = pmisc.tile([128, QCH], F32, tag="m")
                mm(out=z_ps[:C, 0:1], lhsT=c2w_sb, rhs=h_sb,
                   start=True, stop=True)
                nc.scalar.activation(out=eg_sb, in_=z_ps[:C, 0:1],
                                     func=mybir.ActivationFunctionType.Exp,
                                     bias=nc2b_sb, scale=-1.0)
                nc.vector.tensor_scalar_add(gate_sb, eg_sb, 1.0)
                nc.vector.reciprocal(out=gate_sb, in_=gate_sb)
                nc.vector.tensor_scalar_mul(gate_sb, gate_sb, PR)

            pending = None
            for ci, (qs, qn) in enumerate(_chunks()):
                y_ps = py.tile([I + 1, QCH], F32, tag="y")
                for g in range(NGR):
                    # the previous chunk's PE tail goes here, a few groups in,
                    # so its divide chain has finished on DVE by now
                    if g == 4 and pending is not None:
                        pending()
                        pending = None
                    st = pst.tile([128, GK, QCH], F32, tag="st")
                    for j in range(GK):
                        t = GK * g + j
                        mm(out=st[:, j, :qn],
                           lhsT=phi4_sb[:, ts(t, KT)],
                           rhs=theta4_sb[:, ds(qs, qn)],
                           start=True, stop=True)
                    e_t = epool.tile([128, GK, QCH], BF16, tag="e")
                    nc.scalar.activation(out=e_t[:, :, :qn], in_=st[:, :, :qn],
                                         func=mybir.ActivationFunctionType.Exp,
                                         scale=1.0 / (4.0 * TEMP))
                    for j in range(GK):
                        t = GK * g + j
                        mm(out=y_ps[:, :qn],
                           lhsT=gt_sb[:, t, : I + 1],
                           rhs=e_t[:, j, :qn],
                           start=(t == 0), stop=(t == NKV - 1))
                if ci == 0:
                    emit_gate()
                # DVE part of the epilogue now (frees y); PE/DMA deferred
                ysum = work.tile([I + 1, QCH], F32, tag="ysum")
                nc.vector.tensor_copy(out=ysum[:, :qn], in_=y_ps[:, :qn])
                recip = work.tile([1, QCH], F32, tag="recip")
                nc.vector.reciprocal(out=recip[:, :qn],
                                     in_=ysum[I : I + 1, :qn])
                bc = work.tile([I, QCH], F32, tag="bc")
                nc.gpsimd.partition_broadcast(bc[:, :qn], recip[:, :qn])
                yn = work.tile([I, QCH], BF16, tag="yn")
                nc.vector.tensor_mul(yn[:, :qn], ysum[:I, :qn], bc[:, :qn])

                def _tail(qs=qs, qn=qn, yn=yn):
                    o_ps = pmisc.tile([128, QCH], F32, tag="m")
                    mm(out=o_ps[:C, :qn], lhsT=ww_sb, rhs=yn[:, :qn],
                       start=True, stop=True)
                    out_sb = work.tile([C, QCH], F32, tag="out")
                    nc.vector.scalar_tensor_tensor(
                        out=out_sb[:, :qn], in0=o_ps[:C, :qn], scalar=gate_sb,
                        in1=xb_sb[:, ds(qs, qn)],
                        op0=mybir.AluOpType.mult, op1=mybir.AluOpType.add)
                    nc.sync.dma_start(out=out_d[:, ds(qs, qn)],
                                      in_=out_sb[:, :qn])

                pending = _tail
            pending()


def build():
    nc = bacc.Bacc("TRN2", target_bir_lowering=False, debug=False)
    names = {
        "xb": ([C, N], F32), "xbh2": ([128, N], BF16),
        "thw": ([128, 128], BF16), "phw": ([128, 128], BF16),
        "gw": ([128, I], BF16), "ww": ([I, C], BF16),
        "c1w": ([C, NB], F32), "c1b": ([NB, 1], F32),
        "c2w": ([NB, C], F32), "nc2b": ([C, 1], F32),
    }
    dr = {k: nc.dram_tensor(k, shp, dt, kind="ExternalInput").ap()
          for k, (shp, dt) in names.items()}
    out_d = nc.dram_tensor("out", [C, QPC], F32, kind="ExternalOutput").ap()
    with tile.TileContext(nc) as tc:
        _emit(tc, nc, dr, out_d)
    nc.compile()
    return nc


_NC = None


def _get_nc():
    global _NC
    if _NC is None:
        _NC = build()
    return _NC


def make_in_maps(inputs):
    bf = ml_dtypes.bfloat16
    xf = np.ascontiguousarray(np.asarray(inputs["x"], np.float32).reshape(B, C, N))
    thwT = np.asarray(inputs["theta_w"], np.float32).T        # [C, I]
    phwT = np.asarray(inputs["phi_w"], np.float32).T
    gwT = np.asarray(inputs["g_w"], np.float32).T
    shared = {
        "thw": np.ascontiguousarray(np.tile(thwT, (2, 4)) * 0.5).astype(bf),
        "phw": np.ascontiguousarray(np.tile(phwT, (2, 4)) * 0.5).astype(bf),
        "gw": np.ascontiguousarray(np.tile(gwT, (2, 1)) * 0.5).astype(bf),
        "ww": np.ascontiguousarray(np.asarray(inputs["W_w"], np.float32).T).astype(bf),
        "c1w": np.ascontiguousarray(np.asarray(inputs["cg1_w"], np.float32).T),
        "c1b": np.asarray(inputs["cg1_b"], np.float32).reshape(NB, 1).copy(),
        "c2w": np.ascontiguousarray(np.asarray(inputs["cg2_w"], np.float32).T),
        "nc2b": (-np.asarray(inputs["cg2_b"], np.float32)).reshape(C, 1).copy(),
    }
    in_maps = []
    for core in range(NCORES):
        b, q0 = core // CPB, (core % CPB) * QPC
        m = dict(shared)
        xr = np.ascontiguousarray(np.roll(xf[b], -q0, axis=1))
        m["xb"] = xr
        m["xbh2"] = np.ascontiguousarray(np.tile(xr, (2, 1))).astype(bf)
        in_maps.append(m)
    return in_maps


def gather(results):
    y = np.empty((B, C, N), np.float32)
    for core in range(NCORES):
        b, q0 = core // CPB, (core % CPB) * QPC
        y[b][:, q0 : q0 + QPC] = results[core]["out"]
    return y.reshape(B, C, H, W)


def run(inputs, trace=False, **kw):
    res = run_bass_kernel_spmd(_get_nc(), make_in_maps(inputs),
                               core_ids=list(range(NCORES)), trace=trace, **kw)
    return gather(res.results), res


def kernel(**inputs):
    out, _ = run(inputs)
    return out


# revision 18
# speedup vs baseline: 2.5070x; 1.0843x over previous
"""Color-preserving non-local block (dense softmax attention, N=9216, I=32)
distributed over 8 TRN2 NeuronCores.

Sharding: data-parallel over batch B=2 (4 cores per batch) x sequence-parallel
over the N=9216 query rows (2304 rows per core).  Each core receives the full
[C, N] image of its batch (rolled so its query slice starts at column 0 --
softmax over keys is permutation-invariant, so rolling the key axis is free),
computes the projections redundantly, and produces its [C, 2304] output slice.
No collectives are needed.

v3: every matmul uses a full K=128 contraction (K<128 streams at half clock on
this part).  theta/phi are projected with 4x-replicated weight matrices so the
QK matmul contracts over 4 redundant copies (St = 4x scores; the 1/4 folds
into the exp scale for free), x is sent twice-stacked on partitions for the
projections, and PV contracts over the 128-wide kv tile with a ones column
appended to g^T so the softmax denominator accumulates in PSUM row 32.
All matmuls are plain 128x128-mode (no tile_position -> no PE mode-switch
drains).  Per-chunk epilogues are deferred one chunk so the PE never waits on
the divide chain.

  main loop over q chunks (512) x kv tile pairs:
      QK:  2 plain matmuls  St[kv, q] = (phi4 tile)^T theta4     (233 ns each)
      exp: one ScalarE instr per pair: E = exp(St / (4 T)) -> bf16
      PV:  2 plain matmuls  Y[0:33, q] += gt_aug^T E   (PSUM accumulate)
"""

import sys

for _p in ("/opt/trn_rl_repo",):
    if _p not in sys.path:
        sys.path.insert(0, _p)

import numpy as np
import ml_dtypes

import concourse.bass as bass
import concourse.tile as tile
from concourse import bacc, mybir
from concourse.bass import ts, ds
from concourse.bass_utils import run_bass_kernel_spmd

F32 = mybir.dt.float32
BF16 = mybir.dt.bfloat16

B, C, H, W = 2, 64, 96, 96
N = H * W                    # 9216
I = 32                       # inter dim
NB = 16                      # gate bottleneck dim
NCORES = 8
CPB = NCORES // B            # cores per batch = 4
QPC = N // CPB               # 2304 query rows per core
KT = 128                     # kv tile
NKV = N // KT                # 72
GK = 3                       # kv tiles per St/exp group
NGR = NKV // GK              # 24 groups
QCH = 512                    # q chunk (PSUM free dim)
GTS = 34                     # gt free stride (33 used, kept 4B-aligned)
TEMP = 1.5
PR = 0.8


def _chunks():
    out = []
    q = 0
    while q < QPC:
        out.append((q, min(QCH, QPC - q)))
        q += QCH
    return out


def _emit(tc, nc, dr, out_d):
    mm = nc.tensor.matmul
    with (
        tc.tile_pool(name="consts", bufs=1) as consts,
        tc.tile_pool(name="work", bufs=2) as work,
        tc.tile_pool(name="epool", bufs=4) as epool,
    ):
        # ---- persistent SBUF tensors -------------------------------------
        xb_sb = consts.tile([C, N], F32)        # residual + gate path
        xbh2_sb = consts.tile([128, N], BF16)   # x stacked twice on partitions
        wbf_sb = consts.tile([128, 288], BF16)  # bf16 weight blob
        thw_sb = wbf_sb[:, 0:128]               # 0.5 * theta_w^T tiled (2, 4)
        phw_sb = wbf_sb[:, 128:256]             # 0.5 * phi_w^T tiled (2, 4)
        gw_sb = wbf_sb[:, 256:288]              # 0.5 * g_w^T tiled (2, 1)
        wf32_sb = consts.tile([C, 146], F32)    # f32 weight blob
        c1w_sb = wf32_sb[:, 0:NB]
        c1b_sb = wf32_sb[:NB, NB : NB + 1]
        c2w_sb = wf32_sb[:NB, 17:81]
        nc2b_sb = wf32_sb[:, 81:82]
        ww_sb = wf32_sb[:I, 82:146]             # W_w^T (fp32 tail matmul)

        theta4_sb = consts.tile([128, QPC], BF16)   # theta replicated x4
        phi4_sb = consts.tile([128, N], BF16)       # phi replicated x4
        gt_sb = consts.tile([128, NKV, GTS], BF16)  # [kv, tile, i | ones | pad]
        gate_sb = consts.tile([C, 1], F32)
        pool_sb = consts.tile([C, 1], F32)
        h_sb = consts.tile([NB, 1], F32)
        eg_sb = consts.tile([C, 1], F32)

        nc.sync.dma_start(out=wbf_sb, in_=dr["wbf"])
        nc.sync.dma_start(out=xbh2_sb[:, :QPC], in_=dr["xbh2"][:, :QPC])
        nc.sync.dma_start(out=xbh2_sb[:, QPC:], in_=dr["xbh2"][:, QPC:])
        nc.sync.dma_start(out=wf32_sb, in_=dr["wf32"])
        nc.sync.dma_start(out=xb_sb, in_=dr["xb"])

        ones72 = consts.tile([128, NKV], F32)
        nc.vector.memset(ones72, 1.0)
        nc.vector.tensor_copy(out=gt_sb[:, :, I], in_=ones72)

        # ---- prologue projections (all K=128) ----------------------------
        with tc.tile_pool(name="ppsum", bufs=4, space="PSUM") as pp:
            for qs, qn in _chunks():
                pt = pp.tile([128, QCH], F32, tag="pp")
                mm(out=pt[:, :qn], lhsT=thw_sb, rhs=xbh2_sb[:, ds(qs, qn)],
                   start=True, stop=True)
                nc.scalar.copy(out=theta4_sb[:, ds(qs, qn)], in_=pt[:, :qn])
            for c in range(N // QCH):
                pt = pp.tile([128, QCH], F32, tag="pp")
                mm(out=pt, lhsT=phw_sb, rhs=xbh2_sb[:, ts(c, QCH)],
                   start=True, stop=True)
                nc.scalar.copy(out=phi4_sb[:, ts(c, QCH)], in_=pt)
            done = 0
            while done < NKV:
                nt = min(16, NKV - done)
                pt = pp.tile([128, QCH], F32, tag="pp")
                for k in range(nt):
                    t = done + k
                    mm(out=pt[:, ts(k, I)], lhsT=xbh2_sb[:, ts(t, KT)],
                       rhs=gw_sb, start=True, stop=True)
                nc.vector.tensor_copy(
                    out=gt_sb[:, done : done + nt, :I],
                    in_=pt[:, : nt * I].rearrange("p (k i) -> p k i", i=I),
                )
                done += nt

        # ---- main loop ---------------------------------------------------
        with (
            tc.tile_pool(name="pst", bufs=2, space="PSUM") as pst,
            tc.tile_pool(name="py", bufs=1, space="PSUM") as py,
            tc.tile_pool(name="pmisc", bufs=1, space="PSUM") as pmisc,
        ):
            def emit_gate():
                # channel gate; emitted after chunk 0's pairs so its matmuls
                # (which wait on the DVE mean-reduce) never block the PE queue
                # ahead of the main stream
                nc.vector.reduce_sum(out=pool_sb, in_=xb_sb,
                                     axis=mybir.AxisListType.X)
                h_ps = pmisc.tile([128, QCH], F32, tag="m")
                mm(out=h_ps[:NB, 0:1], lhsT=c1w_sb, rhs=pool_sb,
                   start=True, stop=True)
                nc.scalar.activation(out=h_sb, in_=h_ps[:NB, 0:1],
                                     func=mybir.ActivationFunctionType.Relu,
                                     bias=c1b_sb, scale=1.0 / float(N))
                z_ps = pmisc.tile([128, QCH], F32, tag="m")
                mm(out=z_ps[:C, 0:1], lhsT=c2w_sb, rhs=h_sb,
                   start=True, stop=True)
                nc.scalar.activation(out=eg_sb, in_=z_ps[:C, 0:1],
                                     func=mybir.ActivationFunctionType.Exp,
                                     bias=nc2b_sb, scale=-1.0)
                nc.vector.tensor_scalar_add(gate_sb, eg_sb, 1.0)
                nc.vector.reciprocal(out=gate_sb, in_=gate_sb)
                nc.vector.tensor_scalar_mul(gate_sb, gate_sb, PR)

            pending = None
            for ci, (qs, qn) in enumerate(_chunks()):
                y_ps = py.tile([I + 1, QCH], F32, tag="y")
                for g in range(NGR):
                    # the previous chunk's PE tail goes here, a few groups in,
                    # so its divide chain has finished on DVE by now
                    if g == 2 and pending is not None:
                        pending()
                        pending = None
                    st = pst.tile([128, GK, QCH], F32, tag="st")
                    for j in range(GK):
                        t = GK * g + j
                        mm(out=st[:, j, :qn],
                           lhsT=phi4_sb[:, ts(t, KT)],
                           rhs=theta4_sb[:, ds(qs, qn)],
                           start=True, stop=True)
                    e_t = epool.tile([128, GK, QCH], BF16, tag="e")
                    nc.scalar.activation(out=e_t[:, :, :qn], in_=st[:, :, :qn],
                                         func=mybir.ActivationFunctionType.Exp,
                                         scale=1.0 / (4.0 * TEMP))
                    for j in range(GK):
                        t = GK * g + j
                        mm(out=y_ps[:, :qn],
                           lhsT=gt_sb[:, t, : I + 1],
                           rhs=e_t[:, j, :qn],
                           start=(t == 0), stop=(t == NKV - 1))
                if ci == 0:
                    emit_gate()
                # epilogue: copy Y out (frees the bank), W-project the
                # UNNORMALIZED Y (so the PE tail never waits on the divide),
                # and fold 1/denominator into the final DVE pass
                ysum = work.tile([I + 1, QCH], F32, tag="ysum")
                nc.vector.tensor_copy(out=ysum[:, :qn], in_=y_ps[:, :qn])
                recip = work.tile([1, QCH], F32, tag="recip")
                nc.vector.reciprocal(out=recip[:, :qn],
                                     in_=ysum[I : I + 1, :qn])
                bc = work.tile([C, QCH], F32, tag="bc")
                nc.gpsimd.partition_broadcast(bc[:, :qn], recip[:, :qn])

                def _tail(qs=qs, qn=qn, ysum=ysum, bc=bc):
                    o_ps = pmisc.tile([128, QCH], F32, tag="m")
                    mm(out=o_ps[:C, :qn], lhsT=ww_sb, rhs=ysum[:I, :qn],
                       start=True, stop=True)
                    t1 = work.tile([C, QCH], F32, tag="t1")
                    nc.vector.tensor_mul(t1[:, :qn], o_ps[:C, :qn], bc[:, :qn])
                    out_sb = work.tile([C, QCH], F32, tag="out")
                    nc.vector.scalar_tensor_tensor(
                        out=out_sb[:, :qn], in0=t1[:, :qn], scalar=gate_sb,
                        in1=xb_sb[:, ds(qs, qn)],
                        op0=mybir.AluOpType.mult, op1=mybir.AluOpType.add)
                    nc.sync.dma_start(out=out_d[:, ds(qs, qn)],
                                      in_=out_sb[:, :qn])

                pending = _tail
            pending()


def build():
    nc = bacc.Bacc("TRN2", target_bir_lowering=False, debug=False)
    names = {
        "xb": ([C, N], F32), "xbh2": ([128, N], BF16),
        "wbf": ([128, 288], BF16), "wf32": ([C, 146], F32),
    }
    dr = {k: nc.dram_tensor(k, shp, dt, kind="ExternalInput").ap()
          for k, (shp, dt) in names.items()}
    out_d = nc.dram_tensor("out", [C, QPC], F32, kind="ExternalOutput").ap()
    with tile.TileContext(nc) as tc:
        _emit(tc, nc, dr, out_d)
    nc.compile()
    return nc


_NC = None


def _get_nc():
    global _NC
    if _NC is None:
        _NC = build()
    return _NC


def make_in_maps(inputs):
    bf = ml_dtypes.bfloat16
    xf = np.ascontiguousarray(np.asarray(inputs["x"], np.float32).reshape(B, C, N))
    thwT = np.asarray(inputs["theta_w"], np.float32).T        # [C, I]
    phwT = np.asarray(inputs["phi_w"], np.float32).T
    gwT = np.asarray(inputs["g_w"], np.float32).T
    wbf = np.zeros((128, 288), np.float32)
    wbf[:, 0:128] = np.tile(thwT, (2, 4)) * 0.5
    wbf[:, 128:256] = np.tile(phwT, (2, 4)) * 0.5
    wbf[:, 256:288] = np.tile(gwT, (2, 1)) * 0.5
    wf32 = np.zeros((C, 146), np.float32)
    wf32[:, 0:NB] = np.asarray(inputs["cg1_w"], np.float32).T
    wf32[:NB, NB] = np.asarray(inputs["cg1_b"], np.float32)
    wf32[:NB, 17:81] = np.asarray(inputs["cg2_w"], np.float32).T
    wf32[:, 81] = -np.asarray(inputs["cg2_b"], np.float32)
    wf32[:I, 82:146] = np.asarray(inputs["W_w"], np.float32).T
    shared = {"wbf": wbf.astype(bf), "wf32": wf32}
    in_maps = []
    for core in range(NCORES):
        b, q0 = core // CPB, (core % CPB) * QPC
        m = dict(shared)
        xr = np.ascontiguousarray(np.roll(xf[b], -q0, axis=1))
        m["xb"] = xr
        m["xbh2"] = np.ascontiguousarray(np.tile(xr, (2, 1))).astype(bf)
        in_maps.append(m)
    return in_maps


def gather(results):
    y = np.empty((B, C, N), np.float32)
    for core in range(NCORES):
        b, q0 = core // CPB, (core % CPB) * QPC
        y[b][:, q0 : q0 + QPC] = results[core]["out"]
    return y.reshape(B, C, H, W)


def run(inputs, trace=False, **kw):
    res = run_bass_kernel_spmd(_get_nc(), make_in_maps(inputs),
                               core_ids=list(range(NCORES)), trace=trace, **kw)
    return gather(res.results), res


def kernel(**inputs):
    out, _ = run(inputs)
    return out


# revision 19
# speedup vs baseline: 2.5753x; 1.0273x over previous
"""Color-preserving non-local block (dense softmax attention, N=9216, I=32)
distributed over 8 TRN2 NeuronCores.

Sharding: data-parallel over batch B=2 (4 cores per batch) x sequence-parallel
over the N=9216 query rows (2304 rows per core).  Each core receives the full
[C, N] image of its batch (rolled so its query slice starts at column 0 --
softmax over keys is permutation-invariant, so rolling the key axis is free),
computes the projections redundantly, and produces its [C, 2304] output slice.
No collectives are needed.

v3: every matmul uses a full K=128 contraction (K<128 streams at half clock on
this part).  theta/phi are projected with 4x-replicated weight matrices so the
QK matmul contracts over 4 redundant copies (St = 4x scores; the 1/4 folds
into the exp scale for free), x is sent twice-stacked on partitions for the
projections, and PV contracts over the 128-wide kv tile with a ones column
appended to g^T so the softmax denominator accumulates in PSUM row 32.
All matmuls are plain 128x128-mode (no tile_position -> no PE mode-switch
drains).  Per-chunk epilogues are deferred one chunk so the PE never waits on
the divide chain.

  main loop over q chunks (512) x kv tile pairs:
      QK:  2 plain matmuls  St[kv, q] = (phi4 tile)^T theta4     (233 ns each)
      exp: one ScalarE instr per pair: E = exp(St / (4 T)) -> bf16
      PV:  2 plain matmuls  Y[0:33, q] += gt_aug^T E   (PSUM accumulate)
"""

import sys

for _p in ("/opt/trn_rl_repo",):
    if _p not in sys.path:
        sys.path.insert(0, _p)

import numpy as np
import ml_dtypes

import concourse.bass as bass
import concourse.tile as tile
from concourse import bacc, mybir
from concourse.bass import ts, ds
from concourse.bass_utils import run_bass_kernel_spmd

F32 = mybir.dt.float32
BF16 = mybir.dt.bfloat16

B, C, H, W = 2, 64, 96, 96
N = H * W                    # 9216
I = 32                       # inter dim
NB = 16                      # gate bottleneck dim
NCORES = 8
CPB = NCORES // B            # cores per batch = 4
QPC = N // CPB               # 2304 query rows per core
KT = 128                     # kv tile
NKV = N // KT                # 72
GK = 3                       # kv tiles per St/exp group
NGR = NKV // GK              # 24 groups
QCH = 512                    # q chunk (PSUM free dim)
GTS = 34                     # gt free stride (33 used, kept 4B-aligned)
TEMP = 1.5
PR = 0.8


def _chunks():
    out = []
    q = 0
    while q < QPC:
        out.append((q, min(QCH, QPC - q)))
        q += QCH
    return out


def _emit(tc, nc, dr, out_d):
    mm = nc.tensor.matmul
    with (
        tc.tile_pool(name="consts", bufs=1) as consts,
        tc.tile_pool(name="work", bufs=2) as work,
        tc.tile_pool(name="epool", bufs=6) as epool,
    ):
        # ---- persistent SBUF tensors -------------------------------------
        xb_sb = consts.tile([C, N], F32)        # residual + gate path
        xbh2_sb = consts.tile([128, N], BF16)   # x stacked twice on partitions
        wbf_sb = consts.tile([128, 352], BF16)  # bf16 weight blob
        thw_sb = wbf_sb[:, 0:128]               # 0.5 * theta_w^T tiled (2, 4)
        phw_sb = wbf_sb[:, 128:256]             # 0.5 * phi_w^T tiled (2, 4)
        gw_sb = wbf_sb[:, 256:288]              # 0.5 * g_w^T tiled (2, 1)
        ww_sb = wbf_sb[:I, 288:352]             # W_w^T
        wf32_sb = consts.tile([C, 82], F32)     # f32 weight blob
        c1w_sb = wf32_sb[:, 0:NB]
        c1b_sb = wf32_sb[:NB, NB : NB + 1]
        c2w_sb = wf32_sb[:NB, 17:81]
        nc2b_sb = wf32_sb[:, 81:82]

        theta4_sb = consts.tile([128, QPC], BF16)   # theta replicated x4
        phi4_sb = consts.tile([128, N], BF16)       # phi replicated x4
        gt_sb = consts.tile([128, NKV, GTS], BF16)  # [kv, tile, i | ones | pad]
        gate_sb = consts.tile([C, 1], F32)
        pool_sb = consts.tile([C, 1], F32)
        h_sb = consts.tile([NB, 1], F32)
        eg_sb = consts.tile([C, 1], F32)

        nc.sync.dma_start(out=wbf_sb, in_=dr["wbf"])
        nc.sync.dma_start(out=xbh2_sb[:, :QPC], in_=dr["xbh2"][:, :QPC])
        nc.sync.dma_start(out=xbh2_sb[:, QPC:], in_=dr["xbh2"][:, QPC:])
        nc.sync.dma_start(out=xb_sb, in_=dr["xb"])
        nc.sync.dma_start(out=wf32_sb, in_=dr["wf32"])

        ones72 = consts.tile([128, NKV], F32)
        nc.vector.memset(ones72, 1.0)
        nc.vector.tensor_copy(out=gt_sb[:, :, I], in_=ones72)

        # ---- prologue projections (all K=128) ----------------------------
        with tc.tile_pool(name="ppsum", bufs=4, space="PSUM") as pp:
            for qs, qn in _chunks():
                pt = pp.tile([128, QCH], F32, tag="pp")
                mm(out=pt[:, :qn], lhsT=thw_sb, rhs=xbh2_sb[:, ds(qs, qn)],
                   start=True, stop=True)
                nc.scalar.copy(out=theta4_sb[:, ds(qs, qn)], in_=pt[:, :qn])
            for c in range(N // QCH):
                pt = pp.tile([128, QCH], F32, tag="pp")
                mm(out=pt, lhsT=phw_sb, rhs=xbh2_sb[:, ts(c, QCH)],
                   start=True, stop=True)
                nc.scalar.copy(out=phi4_sb[:, ts(c, QCH)], in_=pt)
            done = 0
            while done < NKV:
                nt = min(16, NKV - done)
                pt = pp.tile([128, QCH], F32, tag="pp")
                for k in range(nt):
                    t = done + k
                    mm(out=pt[:, ts(k, I)], lhsT=xbh2_sb[:, ts(t, KT)],
                       rhs=gw_sb, start=True, stop=True)
                nc.vector.tensor_copy(
                    out=gt_sb[:, done : done + nt, :I],
                    in_=pt[:, : nt * I].rearrange("p (k i) -> p k i", i=I),
                )
                done += nt

        # ---- main loop ---------------------------------------------------
        with (
            tc.tile_pool(name="pst", bufs=2, space="PSUM") as pst,
            tc.tile_pool(name="py", bufs=1, space="PSUM") as py,
            tc.tile_pool(name="pmisc", bufs=1, space="PSUM") as pmisc,
        ):
            def emit_gate():
                # channel gate; emitted after chunk 0's pairs so its matmuls
                # (which wait on the DVE mean-reduce) never block the PE queue
                # ahead of the main stream
                nc.vector.reduce_sum(out=pool_sb, in_=xb_sb,
                                     axis=mybir.AxisListType.X)
                h_ps = pmisc.tile([128, QCH], F32, tag="m")
                mm(out=h_ps[:NB, 0:1], lhsT=c1w_sb, rhs=pool_sb,
                   start=True, stop=True)
                nc.scalar.activation(out=h_sb, in_=h_ps[:NB, 0:1],
                                     func=mybir.ActivationFunctionType.Relu,
                                     bias=c1b_sb, scale=1.0 / float(N))
                z_ps = pmisc.tile([128, QCH], F32, tag="m")
                mm(out=z_ps[:C, 0:1], lhsT=c2w_sb, rhs=h_sb,
                   start=True, stop=True)
                nc.scalar.activation(out=eg_sb, in_=z_ps[:C, 0:1],
                                     func=mybir.ActivationFunctionType.Exp,
                                     bias=nc2b_sb, scale=-1.0)
                nc.vector.tensor_scalar_add(gate_sb, eg_sb, 1.0)
                nc.vector.reciprocal(out=gate_sb, in_=gate_sb)
                nc.vector.tensor_scalar_mul(gate_sb, gate_sb, PR)

            pending = None
            for ci, (qs, qn) in enumerate(_chunks()):
                y_ps = py.tile([I + 1, QCH], F32, tag="y")
                for g in range(NGR):
                    # the previous chunk's PE tail goes here, a few groups in,
                    # so its divide chain has finished on DVE by now
                    if g == 6 and pending is not None:
                        pending()
                        pending = None
                    st = pst.tile([128, GK, QCH], F32, tag="st")
                    for j in range(GK):
                        t = GK * g + j
                        mm(out=st[:, j, :qn],
                           lhsT=phi4_sb[:, ts(t, KT)],
                           rhs=theta4_sb[:, ds(qs, qn)],
                           start=True, stop=True)
                    e_t = epool.tile([128, GK, QCH], BF16, tag="e")
                    nc.scalar.activation(out=e_t[:, :, :qn], in_=st[:, :, :qn],
                                         func=mybir.ActivationFunctionType.Exp,
                                         scale=1.0 / (4.0 * TEMP))
                    for j in range(GK):
                        t = GK * g + j
                        mm(out=y_ps[:, :qn],
                           lhsT=gt_sb[:, t, : I + 1],
                           rhs=e_t[:, j, :qn],
                           start=(t == 0), stop=(t == NKV - 1))
                if ci == 0:
                    emit_gate()
                # epilogue: copy Y out (frees the bank), W-project the
                # UNNORMALIZED Y (so the PE tail never waits on the divide),
                # and fold 1/denominator into the final DVE pass
                ysum = work.tile([I, QCH], BF16, tag="ysum")
                nc.vector.tensor_copy(out=ysum[:, :qn], in_=y_ps[:I, :qn])
                d_sb = work.tile([1, QCH], F32, tag="d")
                nc.vector.tensor_copy(out=d_sb[:, :qn], in_=y_ps[I : I + 1, :qn])
                recip = work.tile([1, QCH], F32, tag="recip")
                nc.vector.reciprocal(out=recip[:, :qn], in_=d_sb[:, :qn])
                bc = work.tile([C, QCH], F32, tag="bc")
                nc.gpsimd.partition_broadcast(bc[:, :qn], recip[:, :qn])

                def _tail(qs=qs, qn=qn, ysum=ysum, bc=bc):
                    o_ps = pmisc.tile([128, QCH], F32, tag="m")
                    mm(out=o_ps[:C, :qn], lhsT=ww_sb, rhs=ysum[:, :qn],
                       start=True, stop=True)
                    t1 = work.tile([C, QCH], F32, tag="t1")
                    nc.vector.tensor_mul(t1[:, :qn], o_ps[:C, :qn], bc[:, :qn])
                    out_sb = work.tile([C, QCH], F32, tag="out")
                    nc.vector.scalar_tensor_tensor(
                        out=out_sb[:, :qn], in0=t1[:, :qn], scalar=gate_sb,
                        in1=xb_sb[:, ds(qs, qn)],
                        op0=mybir.AluOpType.mult, op1=mybir.AluOpType.add)
                    nc.sync.dma_start(out=out_d[:, ds(qs, qn)],
                                      in_=out_sb[:, :qn])

                pending = _tail
            pending()


def build():
    nc = bacc.Bacc("TRN2", target_bir_lowering=False, debug=False)
    names = {
        "xb": ([C, N], F32), "xbh2": ([128, N], BF16),
        "wbf": ([128, 352], BF16), "wf32": ([C, 82], F32),
    }
    dr = {k: nc.dram_tensor(k, shp, dt, kind="ExternalInput").ap()
          for k, (shp, dt) in names.items()}
    out_d = nc.dram_tensor("out", [C, QPC], F32, kind="ExternalOutput").ap()
    with tile.TileContext(nc) as tc:
        _emit(tc, nc, dr, out_d)
    nc.compile()
    return nc


_NC = None


def _get_nc():
    global _NC
    if _NC is None:
        _NC = build()
    return _NC


def make_in_maps(inputs):
    bf = ml_dtypes.bfloat16
    xf = np.ascontiguousarray(np.asarray(inputs["x"], np.float32).reshape(B, C, N))
    thwT = np.asarray(inputs["theta_w"], np.float32).T        # [C, I]
    phwT = np.asarray(inputs["phi_w"], np.float32).T
    gwT = np.asarray(inputs["g_w"], np.float32).T
    wbf = np.zeros((128, 352), np.float32)
    wbf[:, 0:128] = np.tile(thwT, (2, 4)) * 0.5
    wbf[:, 128:256] = np.tile(phwT, (2, 4)) * 0.5
    wbf[:, 256:288] = np.tile(gwT, (2, 1)) * 0.5
    wbf[:I, 288:352] = np.asarray(inputs["W_w"], np.float32).T
    wf32 = np.zeros((C, 82), np.float32)
    wf32[:, 0:NB] = np.asarray(inputs["cg1_w"], np.float32).T
    wf32[:NB, NB] = np.asarray(inputs["cg1_b"], np.float32)
    wf32[:NB, 17:81] = np.asarray(inputs["cg2_w"], np.float32).T
    wf32[:, 81] = -np.asarray(inputs["cg2_b"], np.float32)
    shared = {"wbf": wbf.astype(bf), "wf32": wf32}
    in_maps = []
    for core in range(NCORES):
        b, q0 = core // CPB, (core % CPB) * QPC
        m = dict(shared)
        xr = np.ascontiguousarray(np.roll(xf[b], -q0, axis=1))
        m["xb"] = xr
        m["xbh2"] = np.ascontiguousarray(np.tile(xr, (2, 1))).astype(bf)
        in_maps.append(m)
    return in_maps


def gather(results):
    y = np.empty((B, C, N), np.float32)
    for core in range(NCORES):
        b, q0 = core // CPB, (core % CPB) * QPC
        y[b][:, q0 : q0 + QPC] = results[core]["out"]
    return y.reshape(B, C, H, W)


def run(inputs, trace=False, **kw):
    res = run_bass_kernel_spmd(_get_nc(), make_in_maps(inputs),
                               core_ids=list(range(NCORES)), trace=trace, **kw)
    return gather(res.results), res


def kernel(**inputs):
    out, _ = run(inputs)
    return out
